# revision 23
# baseline (speedup 1.0000x reference)
"""Chamfer loss kernel for 8 Trainium2 NeuronCores.

Problem: x, y ~ [B=4, N=8192, 3] fp32.
    d[b,n,m] = ||x_bn||^2 + ||y_bm||^2 - 2 x_bn . y_bm
    loss = mean_b( mean_n min_m d  +  mean_m min_n d )

Sharding: core c -> batch b = c//2, half h = c%2.  Each core runs two
"passes" of a generic [queries x refs] min-distance kernel:
    pass 0: queries = x[b, h*4096:(h+1)*4096], refs = y[b]   (cham_x half)
    pass 1: queries = y[b, h*4096:(h+1)*4096], refs = x[b]   (cham_y half)
Device returns per-query min distances [2, 4096] per core; the host does
the O(B*N) means.

Device algorithm: the whole distance computation is folded into a single
K=5 matmul contraction on the TensorEngine:
    qT rows = [q0, q1, q2, ||q||^2, 1]
    rT rows = [-2*r0, -2*r1, -2*r2, 1, ||r||^2]
    d[p, f] = sum_k qT[k, p] * rT[k, f]
PE emits [128 x 512] distance tiles directly into PSUM; the reduction
(min over refs) runs on VectorE via fused tensor_tensor_reduce over PSUM
bank pairs, optionally with ScalarE copying half the banks to SBUF so
both DVE read ports stay busy.

dtype modes:
  f32r   - float32r matmul (full fp32 bits, 1 cyc/row when free dim >= 256)
  f32    - plain fp32 matmul (4 cyc/row, exact; slow fallback)
  bf16hl - bf16 hi/lo split, K=13 (fast fallback if f32r is inexact on HW)
"""

import functools
import os

import numpy as np

import concourse.bass as bass
import concourse.mybir as mybir
import concourse.tile as tile
from concourse.bass import ts
from concourse.bass_utils import run_bass_kernel_spmd

P = 128          # partitions / queries per tile
F = 512          # matmul free dim = one PSUM bank of fp32
B = 4
N = 8192         # points per cloud (both x and y)
NQ = N // 2      # queries per core per pass
NR = N           # refs per pass
N_CORES = 8

DTYPE_MODE = os.environ.get("CHAMFER_DTYPE", "bf16x3")
REDUCE_MODE = os.environ.get("CHAMFER_REDUCE", "reduce")

FP32_MAX = float(np.finfo(np.float32).max)


def _k_rows(dtype_mode):
    return {"bf16hl": 16, "bf16x3": 24}.get(dtype_mode, 5)


def build_nc(dtype_mode=DTYPE_MODE, reduce_mode=REDUCE_MODE, nq=NQ, nr=NR):
    """Build the SPMD Bass program (same program for all 8 cores)."""
    K = _k_rows(dtype_mode)
    if dtype_mode == "f32r":
        in_dt = mybir.dt.float32r
    elif dtype_mode == "f32":
        in_dt = mybir.dt.float32
    elif dtype_mode == "bf16hl":
        in_dt = mybir.dt.bfloat16
    else:
        raise ValueError(dtype_mode)

    n_qt = nq // P           # query tiles per pass
    n_mt = nr // F           # ref (moving) tiles per pass
    assert n_mt % 2 == 0

    nc = bass.Bass()
    qT_d = nc.dram_tensor("qT", [2, K, nq], in_dt, kind="ExternalInput")
    rT_d = nc.dram_tensor("rT", [2, K, nr], in_dt, kind="ExternalInput")
    mins_d = nc.dram_tensor("mins", [2, nq], mybir.dt.float32,
                            kind="ExternalOutput")

    with tile.TileContext(nc) as tc:
        with (
            tc.tile_pool(name="const", bufs=1) as const_pool,
            tc.tile_pool(name="psum", bufs=8, space="PSUM") as psum_pool,
            tc.tile_pool(name="scratch", bufs=4) as scratch_pool,
            tc.tile_pool(name="partials", bufs=2) as part_pool,
        ):
            qT_sb = const_pool.tile([K, 2 * nq], in_dt)
            rT_sb = const_pool.tile([K, 2 * nr], in_dt)
            out_sb = const_pool.tile([P, 2 * n_qt], mybir.dt.float32)
            dummy = const_pool.tile([P, 1], mybir.dt.float32)

            for p in range(2):
                nc.gpsimd.dma_start(qT_sb[:, ts(p, nq)], qT_d[p, :, :])
                nc.gpsimd.dma_start(rT_sb[:, ts(p, nr)], rT_d[p, :, :])
            # Matmult (via its LDWEIGHTS lowering) can carry at most one
            # sync wait in walrus codegen. Touch each DMA'd region once
            # with a throwaway matmul (one DMA-queue wait each) so real
            # matmuls only ever wait on their PSUM slot release.
            for p in range(2):
                for region, width in ((qT_sb[:, ts(p, nq)], nq),
                                      (rT_sb[:, ts(p, nr)], nr)):
                    ps = psum_pool.tile([P, F], mybir.dt.float32, tag="ps")
                    nc.tensor.matmul(ps[:, :16], region[:, :P],
                                     region[:, :16], start=True, stop=True)
            tc.no_sync_barrier()

            for p in range(2):
                for qt in range(n_qt):
                    lhsT = qT_sb[:, p * nq + qt * P: p * nq + (qt + 1) * P]
                    col = p * n_qt + qt
                    n_parts = n_mt if reduce_mode == "reduce" else n_mt // 2
                    parts = part_pool.tile([P, n_parts], mybir.dt.float32)
                    for j2 in range(n_mt // 2):
                        ps_a = psum_pool.tile([P, F], mybir.dt.float32,
                                              tag="ps")
                        ps_b = psum_pool.tile([P, F], mybir.dt.float32,
                                              tag="ps")
                        nc.tensor.matmul(
                            ps_a[:], lhsT, rT_sb[:, p * nr + (2 * j2) * F:
                                                 p * nr + (2 * j2 + 1) * F],
                            start=True, stop=True)
                        nc.tensor.matmul(
                            ps_b[:], lhsT, rT_sb[:, p * nr + (2 * j2 + 1) * F:
                                                 p * nr + (2 * j2 + 2) * F],
                            start=True, stop=True)
                        if reduce_mode == "reduce":
                            # baseline: per-bank tensor_reduce; parts gets
                            # two columns per j2
                            nc.vector.tensor_reduce(
                                parts[:, 2 * j2: 2 * j2 + 1], ps_a[:],
                                axis=mybir.AxisListType.X,
                                op=mybir.AluOpType.min)
                            nc.vector.tensor_reduce(
                                parts[:, 2 * j2 + 1: 2 * j2 + 2], ps_b[:],
                                axis=mybir.AxisListType.X,
                                op=mybir.AluOpType.min)
                        elif reduce_mode == "ttr":
                            nc.vector.tensor_tensor_reduce(
                                dummy.broadcast_to((P, F)), ps_a[:], ps_b[:],
                                scale=1.0, scalar=FP32_MAX,
                                op0=mybir.AluOpType.min,
                                op1=mybir.AluOpType.min,
                                accum_out=parts[:, j2: j2 + 1])
                        elif reduce_mode == "assist":
                            sc = scratch_pool.tile([P, F], mybir.dt.float32)
                            nc.scalar.copy(sc[:], ps_b[:])
                            nc.vector.tensor_tensor_reduce(
                                dummy.broadcast_to((P, F)), ps_a[:], sc[:],
                                scale=1.0, scalar=FP32_MAX,
                                op0=mybir.AluOpType.min,
                                op1=mybir.AluOpType.min,
                                accum_out=parts[:, j2: j2 + 1])
                        else:
                            raise ValueError(reduce_mode)
                    nc.vector.tensor_reduce(
                        out_sb[:, col: col + 1], parts[:, :n_parts],
                        axis=mybir.AxisListType.X, op=mybir.AluOpType.min)

            # mins[a, t*128 + p] = out_sb[p, a*n_qt + t]
            mins_view = mins_d[:, :].rearrange("a (t p) -> p (a t)", p=P)
            nc.gpsimd.dma_start(mins_view, out_sb[:])

    return nc


def build_nc_raw(dtype_mode=DTYPE_MODE, reduce_mode=REDUCE_MODE, nq=NQ,
                 nr=NR, n_reps=1):
    """Raw-bass variant: explicit semaphores, every instruction carries at
    most ONE sync wait and ONE update (this walrus rejects more)."""
    K = _k_rows(dtype_mode)
    in_dt = {"f32r": mybir.dt.float32r, "f32": mybir.dt.float32,
             "bf16hl": mybir.dt.bfloat16,
             "bf16x3": mybir.dt.bfloat16}[dtype_mode]

    n_qt = nq // P
    n_mt = nr // F
    assert n_mt % 2 == 0
    n_pairs_per_qt = n_mt // 2
    n_pairs = 2 * n_qt * n_pairs_per_qt * n_reps   # both passes x reps

    nc = bass.Bass()
    qT_d = nc.dram_tensor("qT", [2, K, nq], in_dt, kind="ExternalInput")
    rT_d = nc.dram_tensor("rT", [2, K, nr], in_dt, kind="ExternalInput")
    # mins laid out [p, pass*n_qt + t]; host un-permutes (q = t*128 + p)
    mins_d = nc.dram_tensor("mins", [P, 2 * n_qt], mybir.dt.float32,
                            kind="ExternalOutput")

    from contextlib import ExitStack
    ctx = ExitStack()
    qT_sb = ctx.enter_context(nc.sbuf_tensor([K, 2 * nq], in_dt))
    rT_sb = ctx.enter_context(nc.sbuf_tensor([K, 2 * nr], in_dt))
    out_sb = ctx.enter_context(nc.sbuf_tensor([P, 2 * n_qt], mybir.dt.float32))
    dummy = ctx.enter_context(nc.sbuf_tensor([P, 1], mybir.dt.float32))
    parts = ctx.enter_context(nc.sbuf_tensor([P, n_pairs_per_qt],
                                             mybir.dt.float32))
    scratch = [ctx.enter_context(
        nc.sbuf_tensor(f"scratch{i}", [P, F], mybir.dt.float32))
        for i in range(4)]
    # four 2-bank tensors: each matmul writes one half, DVE reduces both
    # halves (1024 elems) in a single standard tensor_reduce
    if reduce_mode == "reduce4":
        psum4 = [ctx.enter_context(
            nc.psum_tensor(f"psum4_{i}", [P, 4 * F], mybir.dt.float32))
            for i in range(2)]
    else:
        psum = [ctx.enter_context(
            nc.psum_tensor(f"psum{i}", [P, 2 * F], mybir.dt.float32))
            for i in range(4)]

    dma_in = ctx.enter_context(nc.semaphore("dma_in"))
    dma_out = ctx.enter_context(nc.semaphore("dma_out"))
    pe_sem = ctx.enter_context(nc.semaphore("pe_sem"))
    act_sem = ctx.enter_context(nc.semaphore("act_sem"))
    dve_sem = ctx.enter_context(nc.semaphore("dve_sem"))
    dve_done = ctx.enter_context(nc.semaphore("dve_done"))

    assist = reduce_mode == "assist"

    def pair_slices(t):
        """t = global pair index -> (pass, qtile, pair-in-qtile)."""
        pss, rem = divmod(t % (2 * n_qt * n_pairs_per_qt),
                          n_qt * n_pairs_per_qt)
        qt, j2 = divmod(rem, n_pairs_per_qt)
        return pss, qt, j2

    if reduce_mode == "reduce4":
        # groups of 4 banks: one matmul-quad + one [128,2048] reduce
        n_groups = n_pairs // 2          # total quad-groups
        ngq = n_pairs_per_qt // 2        # groups per q-tile

        def after_ttr(g):
            return g + g // ngq + 1

        def after_red(k):
            return (ngq + 1) * (k + 1)

        total_dve = after_red(2 * n_qt * n_reps - 1)
    else:
        npq = n_pairs_per_qt

        def after_ttr(t):
            # dve_sem value once reduce t completes (q-tile tails interleave)
            return t + t // npq + 1

        def after_red(k):
            return (npq + 1) * (k + 1)

        total_dve = after_red(2 * n_qt * n_reps - 1)

    with nc.Block() as block:

        @block.gpsimd
        def _(eng):
            for p in range(2):
                eng.dma_start(qT_sb[:, ts(p, nq)],
                              qT_d[p, :, :]).then_inc(dma_in, 16)
                eng.dma_start(rT_sb[:, ts(p, nr)],
                              rT_d[p, :, :]).then_inc(dma_in, 16)
            eng.wait_ge(dve_sem, total_dve)
            eng.dma_start(mins_d[:, :], out_sb[:]).then_inc(dma_out, 16)
            eng.wait_ge(dma_out, 16)

        @block.tensor
        def _(eng):
            eng.wait_ge(dma_in, 64)
            if reduce_mode == "reduce4":
                for g in range(n_groups):
                    for half in range(2):
                        t = 2 * g + half
                        pss, qt, j2 = pair_slices(t)
                        lhsT = qT_sb[:, pss * nq + qt * P:
                                     pss * nq + (qt + 1) * P]
                        pt = psum4[g % 2]
                        for s in range(2):
                            rr = rT_sb[:, pss * nr + (2 * j2 + s) * F:
                                       pss * nr + (2 * j2 + s + 1) * F]
                            off = (2 * half + s) * F
                            mm = nc.tensor.matmul(pt[:, off:off + F], lhsT,
                                                  rr, start=True, stop=True)
                            if g >= 2 and half == 0 and s == 0:
                                mm._wait_ge(dve_sem, after_ttr(g - 2))
                            if half == 1 and s == 1:
                                mm.then_inc(pe_sem, 1)
                return
            for t in range(n_pairs):
                pss, qt, j2 = pair_slices(t)
                lhsT = qT_sb[:, pss * nq + qt * P: pss * nq + (qt + 1) * P]
                ra = rT_sb[:, pss * nr + (2 * j2) * F:
                           pss * nr + (2 * j2 + 1) * F]
                rb = rT_sb[:, pss * nr + (2 * j2 + 1) * F:
                           pss * nr + (2 * j2 + 2) * F]
                pt = psum[t % 4]
                mm = nc.tensor.matmul(pt[:, :F], lhsT, ra,
                                      start=True, stop=True)
                if t >= 4:
                    # slot reused from pair t-4: its reduce must be done
                    mm._wait_ge(dve_sem, after_ttr(t - 4))
                nc.tensor.matmul(pt[:, F:], lhsT, rb,
                                 start=True, stop=True).then_inc(pe_sem, 1)

        @block.vector
        def _(eng):
            if reduce_mode == "reduce4":
                for g in range(n_groups):
                    pss, qt, j2 = pair_slices(2 * g)
                    jg = j2 // 2
                    k = g // ngq
                    if jg == 0 and k > 0:
                        eng.wait_ge(dve_sem, after_red(k - 1))
                    nc.vector.tensor_reduce(
                        parts[:, jg: jg + 1], psum4[g % 2][:, :],
                        axis=mybir.AxisListType.X,
                        op=mybir.AluOpType.min)._wait_ge(
                        pe_sem, g + 1).then_inc(dve_sem, 1)
                    if jg == ngq - 1:
                        col = pss * n_qt + qt
                        nc.vector.tensor_reduce(
                            out_sb[:, col: col + 1], parts[:, :ngq],
                            axis=mybir.AxisListType.X,
                            op=mybir.AluOpType.min)._wait_ge(
                            dve_sem, after_ttr(g)).then_inc(dve_sem, 1)
                return
            for t in range(n_pairs):
                pss, qt, j2 = pair_slices(t)
                k = t // npq
                pt = psum[t % 4]
                if j2 == 0 and k > 0:
                    # WAR on parts vs previous q-tile's reduce
                    eng.wait_ge(dve_sem, after_red(k - 1))
                nc.vector.tensor_reduce(
                    parts[:, j2: j2 + 1], pt[:, :],
                    axis=mybir.AxisListType.X,
                    op=mybir.AluOpType.min)._wait_ge(
                    pe_sem, t + 1).then_inc(dve_sem, 1)
                if j2 == npq - 1:
                    col = pss * n_qt + qt
                    nc.vector.tensor_reduce(
                        out_sb[:, col: col + 1], parts[:],
                        axis=mybir.AxisListType.X,
                        op=mybir.AluOpType.min)._wait_ge(
                        dve_sem, after_ttr(t)).then_inc(dve_sem, 1)

    ctx.close()
    return nc


def _aug_f32(q, r):
    """q [nq,3], r [nr,3] fp32 -> qT [5,nq], rT [5,nr] fp32."""
    q = q.astype(np.float32)
    r = r.astype(np.float32)
    q2 = np.sum(q * q, axis=1, dtype=np.float32)
    r2 = np.sum(r * r, axis=1, dtype=np.float32)
    qT = np.stack([q[:, 0], q[:, 1], q[:, 2], q2,
                   np.ones_like(q2)], axis=0)
    rT = np.stack([-2.0 * r[:, 0], -2.0 * r[:, 1], -2.0 * r[:, 2],
                   np.ones_like(r2), r2], axis=0)
    return qT.astype(np.float32), rT.astype(np.float32)


def _aug_bf16hl(q, r):
    """bf16 hi/lo split, K=13 rows."""
    import ml_dtypes
    bf16 = ml_dtypes.bfloat16

    def split(v):
        hi = v.astype(bf16).astype(np.float32)
        lo = (v - hi).astype(bf16).astype(np.float32)
        return hi, lo

    q = q.astype(np.float32)
    r = r.astype(np.float32)
    q2 = np.sum(q * q, axis=1, dtype=np.float32)
    r2 = np.sum(r * r, axis=1, dtype=np.float32)
    qh, ql = split(q.T)        # [3, nq] each
    rh, rl = split(r.T)        # [3, nr]
    q2h, q2l = split(q2)
    r2h, r2l = split(r2)
    ones_q = np.ones_like(q2)
    ones_r = np.ones_like(r2)
    # d = sum_i [ xh*(-2yh) + xh*(-2yl) + xl*(-2yh) + xl*(-2yl) ]
    #     + x2h + x2l + y2h + y2l
    qT = np.concatenate([qh, qh, ql, ql,
                         q2h[None], q2l[None], ones_q[None], ones_q[None]],
                        axis=0)
    rT = np.concatenate([-2.0 * rh, -2.0 * rl, -2.0 * rh, -2.0 * rl,
                         ones_r[None], ones_r[None], r2h[None], r2l[None]],
                        axis=0)
    return qT.astype(bf16), rT.astype(bf16)


def _unpermute_mins(arr, n_qt=NQ // P):
    """[128, 2*n_qt] device layout -> [2, n_qt*128] per-query mins."""
    out = np.empty((2, n_qt * P), np.float32)
    for a in range(2):
        out[a] = arr[:, a * n_qt:(a + 1) * n_qt].T.reshape(-1)
    return out


def _aug_bf16x3(q, r):
    """3-level bf16 split, K=24 rows; d accurate to ~1e-6 abs."""
    import ml_dtypes
    bf16 = ml_dtypes.bfloat16

    def split3(v):
        h = v.astype(bf16).astype(np.float32)
        m = (v - h).astype(bf16).astype(np.float32)
        l = (v - h - m).astype(bf16).astype(np.float32)
        return h, m, l

    q = q.astype(np.float32)
    r = r.astype(np.float32)
    q2 = np.sum(q * q, axis=1, dtype=np.float32)
    r2 = np.sum(r * r, axis=1, dtype=np.float32)
    qh, qm, ql = split3(q.T)
    rh, rm, rl = split3(r.T)
    q2h, q2m, q2l = split3(q2)
    r2h, r2m, r2l = split3(r2)
    on = np.ones_like(q2)
    om = np.ones_like(r2)
    # products kept: hh, hm, mh, mm, hl, lh  (ml/lm/ll < 2^-26)
    qT = np.concatenate([qh, qh, qm, qm, qh, ql,
                         q2h[None], q2m[None], q2l[None],
                         on[None], on[None], on[None]], axis=0)
    rT = np.concatenate([-2*rh, -2*rm, -2*rh, -2*rm, -2*rl, -2*rh,
                         om[None], om[None], om[None],
                         r2h[None], r2m[None], r2l[None]], axis=0)
    return qT.astype(bf16), rT.astype(bf16)


def _prep_in_maps(x, y, dtype_mode=DTYPE_MODE):
    aug = {"bf16hl": _aug_bf16hl, "bf16x3": _aug_bf16x3}.get(
        dtype_mode, _aug_f32)
    in_maps = []
    for c in range(N_CORES):
        b, h = divmod(c, 2)
        xq = x[b, h * NQ:(h + 1) * NQ]
        yq = y[b, h * NQ:(h + 1) * NQ]
        qT0, rT0 = aug(xq, y[b])
        qT1, rT1 = aug(yq, x[b])
        in_maps.append({
            "qT": np.stack([qT0, qT1], axis=0),
            "rT": np.stack([rT0, rT1], axis=0),
        })
    return in_maps


@functools.lru_cache(maxsize=2)
def _cached_nc(dtype_mode, reduce_mode):
    return build_nc_raw(dtype_mode, reduce_mode)


def _stub_ntff_hook():
    """antenv.axon_hooks is absent in this client; stub it so trace=True
    degrades to a plain run instead of crashing."""
    import sys
    import types
    if "antenv.axon_hooks" not in sys.modules:
        m = types.ModuleType("antenv.axon_hooks")
        m.get_axon_ntff_profile_hook = lambda: None
        sys.modules["antenv.axon_hooks"] = m


def run_device(x, y, dtype_mode=DTYPE_MODE, reduce_mode=REDUCE_MODE,
               trace=False, **kw):
    """Run the device kernel; returns (mins [8,2,4096], BassKernelResults)."""
    if trace:
        try:
            from antenv.axon_hooks import get_axon_ntff_profile_hook  # noqa
        except ImportError:
            _stub_ntff_hook()
    nc = _cached_nc(dtype_mode, reduce_mode)
    in_maps = _prep_in_maps(x, y, dtype_mode)
    res = run_bass_kernel_spmd(nc, in_maps, list(range(N_CORES)),
                               trace=trace, **kw)
    mins = np.stack([_unpermute_mins(res.results[c]["mins"])
                     for c in range(N_CORES)], axis=0)
    return mins, res


def finish(mins):
    """mins [8, 2, 4096] -> scalar loss (host, float64 accumulate)."""
    total = 0.0
    for b in range(B):
        cham_x = np.concatenate([mins[2 * b, 0], mins[2 * b + 1, 0]])
        cham_y = np.concatenate([mins[2 * b, 1], mins[2 * b + 1, 1]])
        total += cham_x.mean(dtype=np.float64) + cham_y.mean(dtype=np.float64)
    return np.float32(total / B)


def kernel(x, y):
    x = np.asarray(x, dtype=np.float32)
    y = np.asarray(y, dtype=np.float32)
    mins, _ = run_device(x, y)
    return finish(mins)
